# revision 22
# baseline (speedup 1.0000x reference)
"""Trainium2 kernel for nn_ClsSegLoss (cls BCE + masked dice seg loss).

Strategy (data-parallel over batch, 8 NeuronCores):
  - cls BCE needs only predict_cls/labels (64 floats) -> host.
  - seg dice only involves samples with predict_cls >= 0.5 ("selected"):
      label==1 selected samples need pg=sum(sig*m), pp=sum(sig^2),
               gg=sum(m)  (mask binary -> == sum(m^2))
      label!=1 selected samples need only psum=sum(sig)
      unselected samples contribute nothing.
    predict_cls/labels are host-visible, so kernel() builds the exact work
    list per call and ships only the needed samples, fp16-cast (masks are
    exactly {0,1} -> fp16 lossless; fp16 logits add ~1e-5 rel err to the
    262144-element sums).
  - Per core: K1 "full" samples (seg+mask, ACT sigmoid -> DVE
    scalar_tensor_tensor pg/pp + tensor_scalar gg, all with fused fp32
    accum_out) and K0 "sig-only" samples (ACT sigmoid with accum_out).
    Per-partition partials [128, 1] land in accumulator tiles, DMA'd out;
    the final 128-way sums + dice/BCE math run on host in float64.
  - This walrus build rejects instructions carrying more than one sync
    wait; _split_excess_waits() moves surplus waits onto same-engine NoOps
    inserted just before (identical semantics on in-order sequencers).
"""

import sys

import numpy as np

for _p in ("/opt/trn_rl_repo",):
    if _p not in sys.path:
        sys.path.insert(0, _p)

import concourse.bass as bass
import concourse.tile as tile
from concourse import mybir
from concourse.bass_utils import run_bass_kernel_spmd

B, C, H, W = 64, 1, 512, 512
N_CORES = 8
N = C * H * W  # elements per sample = 262144
P = 128  # SBUF partitions
F = N // P  # free dim per sample tile = 2048

_F32 = mybir.dt.float32
_F16 = mybir.dt.float16

_split_ctr = [0]


def _split_excess_waits(nc: bass.Bass, max_waits: int = 1) -> bass.Bass:
    """Move surplus sync waits onto same-engine NoOps (walrus allows only
    one wait per instruction in this build)."""
    for bb in nc.main_func.blocks:
        insts = bb.instructions
        new = []
        changed = False
        for ins in insts:
            si = getattr(ins, "sync_info", None)
            waits = list(si.on_wait) if (si is not None and si.on_wait) else []
            if len(waits) > max_waits:
                keep = waits[-max_waits:]
                extra = waits[:-max_waits]
                for k in range(0, len(extra), max_waits):
                    chunk = extra[k : k + max_waits]
                    _split_ctr[0] += 1
                    new.append(
                        mybir.InstNoOp(
                            name=f"ant_wait_split_{_split_ctr[0]}",
                            engine=ins.engine,
                            ins=[],
                            outs=[],
                            sync_info=mybir.SyncInfo(on_wait=chunk, on_update=[]),
                        )
                    )
                ins.sync_info = mybir.SyncInfo(
                    on_wait=keep, on_update=list(si.on_update)
                )
                changed = True
            new.append(ins)
        if changed:
            insts[:] = new
    return nc


def _build_nc(K1: int, K0: int) -> bass.Bass:
    """Per-core program: K1 full samples (pg/pp/gg), K0 sig-only (psum)."""
    nc = bass.Bass()
    AF = mybir.ActivationFunctionType
    OP = mybir.AluOpType

    seg_full = msk_full = seg_sig = None
    if K1 > 0:
        seg_full = nc.declare_dram_parameter("seg_full", [K1, P, F], _F16, False)[:]
        msk_full = nc.declare_dram_parameter("msk_full", [K1, P, F], _F16, False)[:]
    if K0 > 0:
        seg_sig = nc.declare_dram_parameter("seg_sig", [K0, P, F], _F16, False)[:]
    # res_v[0, 2j:2j+2] = (pp_j, gg_j), already fully reduced
    res_v = (
        nc.declare_dram_parameter("res_v", [1, 2 * K1], _F32, True) if K1 else None
    )
    # res_pg[:, j] = per-partition pg partials (STT accum)
    res_pg = nc.declare_dram_parameter("res_pg", [P, K1], _F32, True) if K1 else None
    # res_a[:, j] = per-partition psum partials (sig-only samples)
    res_a = nc.declare_dram_parameter("res_a", [P, K0], _F32, True) if K0 else None

    NCHUNK = 4  # matmul free-dim chunks (N=512 max per PSUM bank)
    CW = F // NCHUNK

    with tile.TileContext(nc) as tc:
        with (
            tc.tile_pool(name="seg_p", bufs=4) as seg_p,
            tc.tile_pool(name="msk_p", bufs=4) as msk_p,
            tc.tile_pool(name="sig_p", bufs=3) as sig_p,
            tc.tile_pool(name="dmp_p", bufs=3) as dmp_p,
            tc.tile_pool(name="acc_p", bufs=1) as acc_p,
            tc.tile_pool(name="ps_p", bufs=3, space="PSUM") as ps_p,
        ):
            acc_v = acc_p.tile([1, 2 * K1], _F32, name="acc_v") if K1 else None
            acc_pg = acc_p.tile([P, K1], _F32, name="acc_pg") if K1 else None
            acc_a = acc_p.tile([P, K0], _F32, name="acc_a") if K0 else None
            ones = None
            if K1:
                ones = acc_p.tile([P, 1], _F16, name="ones")
                nc.vector.memset(ones, 1.0)
            # fulls first: DVE's dependency chain (sigmoid -> STT/TT -> PE ->
            # reduce) never waits behind sig-only sigmoids on ACT, and the
            # kernel tail is a short sig-only chain.
            # One coalesced DMA per tensor group: fewer issue slots on the
            # Sync sequencer and late samples' data arrives far earlier.
            sfull_t = mfull_t = ssig_t = None
            if K1:
                sfull_t = seg_p.tile([P, K1, F], _F16, tag="sf")
                mfull_t = msk_p.tile([P, K1, F], _F16, tag="mf")
                nc.sync.dma_start(out=sfull_t, in_=seg_full.rearrange("j p f -> p j f"))
                nc.sync.dma_start(out=mfull_t, in_=msk_full.rearrange("j p f -> p j f"))
            if K0:
                ssig_t = seg_p.tile([P, K0, F], _F16, tag="ss")
                nc.sync.dma_start(out=ssig_t, in_=seg_sig.rearrange("j p f -> p j f"))
            order = [("full", j) for j in range(K1)] + [
                ("sig", j) for j in range(K0)
            ]
            for kind, j in order:
                if kind == "full":
                    s = sfull_t[:, j, :]
                    m = mfull_t[:, j, :]
                    g = sig_p.tile([P, F], _F16, tag="g")
                    nc.scalar.activation(g, s, AF.Sigmoid)
                    # pg = sum(g*m): one fused DVE pass with fp32 accum
                    d0 = dmp_p.tile([P, F], _F16, tag="d0")
                    nc.vector.scalar_tensor_tensor(
                        out=d0, in0=g, scalar=1.0, in1=m,
                        op0=OP.mult, op1=OP.mult,
                        accum_out=acc_pg[:, j : j + 1],
                    )
                    # pp: g*g product on DVE (fp16 TT -> 2x), reduced on PE
                    d1 = dmp_p.tile([P, F], _F16, tag="d1")
                    nc.vector.tensor_mul(d1, g, g)
                    # TensorE: ones^T @ src accumulates partition-sums into
                    # PSUM rows [1, CW]; (pp, gg) x NCHUNK chunks.
                    ps = ps_p.tile([1, 2, CW], _F32, tag="ps")
                    # q=0: gg from m (gated only on the mask DMA, runs early)
                    # q=1: pp from d1 (gated on the TT product)
                    for q, src in enumerate((m, d1)):
                        srcv = src.rearrange("p (c w) -> p c w", c=NCHUNK)
                        for c in range(NCHUNK):
                            nc.tensor.matmul(
                                ps[:, q, :],
                                ones,
                                srcv[:, c, :],
                                start=(c == 0),
                                stop=(c == NCHUNK - 1),
                            )
                        # stage 2 per quantity so the gg reduce overlaps the
                        # pp matmuls: [1, CW] -> [1, 1]
                        nc.vector.tensor_reduce(
                            acc_v[:, 2 * j + q : 2 * j + q + 1],
                            ps[:, q, :],
                            axis=mybir.AxisListType.X,
                            op=OP.add,
                        )
                else:
                    s = ssig_t[:, j, :]
                    g = sig_p.tile([P, F], _F16, tag="g")
                    # psum = sum(sigmoid(s))
                    nc.scalar.activation(
                        g, s, AF.Sigmoid, accum_out=acc_a[:, j : j + 1]
                    )
            if K1:
                nc.sync.dma_start(out=res_v[:], in_=acc_v)
                nc.sync.dma_start(out=res_pg[:], in_=acc_pg)
            if K0:
                nc.sync.dma_start(out=res_a[:], in_=acc_a)
    return _split_excess_waits(nc)


def _build_nc_raw(K1: int, K0: int) -> bass.Bass:
    """Raw-Block variant: no TileContext entry/exit barriers, DMAs issue as
    the very first sync-engine instructions, explicit semaphores."""
    from contextlib import ExitStack

    nc = bass.Bass()
    AF = mybir.ActivationFunctionType
    OP = mybir.AluOpType

    seg_full = msk_full = seg_sig = None
    if K1 > 0:
        seg_full = nc.declare_dram_parameter("seg_full", [K1, P, F], _F16, False)[:]
        msk_full = nc.declare_dram_parameter("msk_full", [K1, P, F], _F16, False)[:]
    if K0 > 0:
        seg_sig = nc.declare_dram_parameter("seg_sig", [K0, P, F], _F16, False)[:]
    res_v = (
        nc.declare_dram_parameter("res_v", [1, 2 * K1], _F32, True) if K1 else None
    )
    res_pg = nc.declare_dram_parameter("res_pg", [P, K1], _F32, True) if K1 else None
    res_a = nc.declare_dram_parameter("res_a", [P, K0], _F32, True) if K0 else None

    NCHUNK = 4
    CW = F // NCHUNK

    order = []
    for j in range(max(K1, K0)):
        if j < K1:
            order.append(("full", j))
        if j < K0:
            order.append(("sig", j))
    # order position of full j's sigmoid (act sem value after it = pos+1)
    act_after = {}
    for k, (kind, j) in enumerate(order):
        act_after[(kind, j)] = k + 1

    with ExitStack() as ctx:
        s_sl = [ctx.enter_context(nc.sbuf_tensor(f"s{i}", [P, F], _F16)) for i in range(3)]
        m_sl = [ctx.enter_context(nc.sbuf_tensor(f"m{i}", [P, F], _F16)) for i in range(3)]
        g_sl = [ctx.enter_context(nc.sbuf_tensor(f"g{i}", [P, F], _F16)) for i in range(2)]
        gd = ctx.enter_context(nc.sbuf_tensor("gd", [P, F], _F16))
        d0 = ctx.enter_context(nc.sbuf_tensor("d0", [P, F], _F16))
        d1_sl = [ctx.enter_context(nc.sbuf_tensor(f"d1{i}", [P, F], _F16)) for i in range(2)]
        ones = ctx.enter_context(nc.sbuf_tensor("ones", [P, 1], _F16))
        acc_v = (
            ctx.enter_context(nc.sbuf_tensor("acc_v", [1, 2 * K1], _F32)) if K1 else None
        )
        acc_pg = ctx.enter_context(nc.sbuf_tensor("acc_pg", [P, K1], _F32)) if K1 else None
        acc_a = ctx.enter_context(nc.sbuf_tensor("acc_a", [P, K0], _F32)) if K0 else None
        ps_sl = [
            ctx.enter_context(nc.psum_tensor(f"ps{i}", [1, 2, CW], _F32)) for i in range(2)
        ]
        sem_dma_s = ctx.enter_context(nc.semaphore("dma_s"))
        sem_dma_m = ctx.enter_context(nc.semaphore("dma_m"))
        sem_act = ctx.enter_context(nc.semaphore("act"))
        sem_dve = ctx.enter_context(nc.semaphore("dve"))
        sem_pe = ctx.enter_context(nc.semaphore("pe"))
        sem_out = ctx.enter_context(nc.semaphore("outd"))
        block = ctx.enter_context(nc.Block())

        # dve counter bookkeeping (memset=1; per full: STT, TT; st2 deferred)
        dve_stt, dve_tt, dve_st2 = {}, {}, {}
        v = 1
        for f in range(K1):
            v += 1
            dve_stt[f] = v
            v += 1
            dve_tt[f] = v
            if f >= 1:
                v += 1
                dve_st2[f - 1] = v
        if K1:
            v += 1
            dve_st2[K1 - 1] = v
        dve_final = v

        @block.sync
        def _(sync):
            fcnt = 0
            for k, (kind, j) in enumerate(order):
                if k >= 3:
                    sync.wait_ge(sem_act, k - 2)  # slot tenant k-3's sigmoid done
                src = seg_full[j] if kind == "full" else seg_sig[j]
                sync.dma_start(out=s_sl[k % 3][:], in_=src).then_inc(sem_dma_s, 16)
                if kind == "full":
                    f = fcnt
                    if f >= 3:
                        sync.wait_ge(sem_dve, dve_stt[f - 3])
                        sync.wait_ge(sem_pe, f - 2)
                    sync.dma_start(out=m_sl[f % 3][:], in_=msk_full[j]).then_inc(
                        sem_dma_m, 16
                    )
                    fcnt += 1
            n_out = 0
            if K1:
                sync.wait_ge(sem_dve, dve_final)
                sync.dma_start(out=res_v[:], in_=acc_v[:]).then_inc(sem_out, 16)
                sync.dma_start(out=res_pg[:], in_=acc_pg[:]).then_inc(sem_out, 16)
                n_out += 2
            if K0:
                sync.wait_ge(sem_act, len(order))
                sync.dma_start(out=res_a[:], in_=acc_a[:]).then_inc(sem_out, 16)
                n_out += 1
            sync.wait_ge(sem_out, 16 * n_out)

        @block.scalar
        def _(scalar):
            fcnt = 0
            for k, (kind, j) in enumerate(order):
                scalar.wait_ge(sem_dma_s, 16 * (k + 1))
                if kind == "full":
                    f = fcnt
                    if f >= 2:
                        scalar.wait_ge(sem_dve, dve_tt[f - 2])  # g slot reuse
                    scalar.activation(
                        g_sl[f % 2][:], s_sl[k % 3][:], AF.Sigmoid
                    ).then_inc(sem_act, 1)
                    fcnt += 1
                else:
                    scalar.activation(
                        gd[:], s_sl[k % 3][:], AF.Sigmoid,
                        accum_out=acc_a[:, j : j + 1],
                    ).then_inc(sem_act, 1)

        if K1:

            @block.vector
            def _(vector):
                vector.memset(ones[:], 1.0)
                nc.vector.nop().then_inc(sem_dve, 1)
                for f in range(K1):
                    vector.wait_ge(sem_act, act_after[("full", f)])
                    vector.wait_ge(sem_dma_m, 16 * (f + 1))
                    vector.scalar_tensor_tensor(
                        out=d0[:], in0=g_sl[f % 2][:], scalar=1.0,
                        in1=m_sl[f % 3][:], op0=OP.mult, op1=OP.mult,
                        accum_out=acc_pg[:, f : f + 1],
                    ).then_inc(sem_dve, 1)
                    if f >= 2:
                        vector.wait_ge(sem_pe, f - 1)  # d1 slot reuse
                    vector.tensor_mul(
                        d1_sl[f % 2][:], g_sl[f % 2][:], g_sl[f % 2][:]
                    ).then_inc(sem_dve, 1)
                    if f >= 1:
                        vector.wait_ge(sem_pe, f)
                        vector.tensor_reduce(
                            acc_v[:, 2 * (f - 1) : 2 * f],
                            ps_sl[(f - 1) % 2][:],
                            axis=mybir.AxisListType.X,
                            op=OP.add,
                        ).then_inc(sem_dve, 1)
                f = K1 - 1
                vector.wait_ge(sem_pe, K1)
                vector.tensor_reduce(
                    acc_v[:, 2 * f : 2 * f + 2],
                    ps_sl[f % 2][:],
                    axis=mybir.AxisListType.X,
                    op=OP.add,
                ).then_inc(sem_dve, 1)

            @block.tensor
            def _(tensor):
                for f in range(K1):
                    tensor.wait_ge(sem_dve, dve_tt[f])  # d1 ready (implies ones)
                    tensor.wait_ge(sem_dma_m, 16 * (f + 1))
                    for q, src in enumerate((d1_sl[f % 2], m_sl[f % 3])):
                        for c in range(NCHUNK):
                            mm = tensor.matmul(
                                ps_sl[f % 2][:1, q, :],
                                ones[:],
                                src[:, c * CW : (c + 1) * CW],
                                start=(c == 0),
                                stop=(c == NCHUNK - 1),
                            )
                            if q == 1 and c == NCHUNK - 1:
                                mm.then_inc(sem_pe, 1)

    return _split_excess_waits(nc)


_NC_CACHE: dict = {}
_RAW_OK: dict = {}


def _get_nc(K1: int, K0: int, raw: bool) -> bass.Bass:
    key = (K1, K0, raw)
    if key not in _NC_CACHE:
        _NC_CACHE[key] = (
            _build_nc_raw(K1, K0) if raw else _build_nc(K1, K0)
        )
    return _NC_CACHE[key]


def _device_sums(seg16, msk16, full_lists, sig_lists, K1, K0, **spmd_kwargs):
    """seg16/msk16: [B, P, F] fp16 views. full_lists/sig_lists: per-core
    sample-index lists (len <= K1/K0).  Returns dicts idx->float64 sums and
    the raw BassKernelResults."""
    in_maps = []
    for c in range(N_CORES):
        im = {}
        if K1:
            sf = np.zeros((K1, P, F), np.float16)
            mf = np.zeros((K1, P, F), np.float16)
            for j, idx in enumerate(full_lists[c]):
                sf[j] = seg16[idx]
                mf[j] = msk16[idx]
            im["seg_full"] = sf
            im["msk_full"] = mf
        if K0:
            ss = np.zeros((K0, P, F), np.float16)
            for j, idx in enumerate(sig_lists[c]):
                ss[j] = seg16[idx]
            im["seg_sig"] = ss
        in_maps.append(im)
    use_raw = _RAW_OK.get((K1, K0), False)
    try:
        out = run_bass_kernel_spmd(
            _get_nc(K1, K0, use_raw), in_maps, list(range(N_CORES)), **spmd_kwargs
        )
    except Exception:
        if not use_raw:
            raise
        _RAW_OK[(K1, K0)] = False
        out = run_bass_kernel_spmd(
            _get_nc(K1, K0, False), in_maps, list(range(N_CORES)), **spmd_kwargs
        )
    pg, pp, gg, psum = {}, {}, {}, {}
    for c in range(N_CORES):
        if K1:
            rv = np.asarray(out.results[c]["res_v"], dtype=np.float64)
            rpg = np.asarray(out.results[c]["res_pg"], dtype=np.float64)
            for j, idx in enumerate(full_lists[c]):
                pg[idx] = rpg[:, j].sum()
                gg[idx] = rv[0, 2 * j]
                pp[idx] = rv[0, 2 * j + 1]
        if K0:
            ra = np.asarray(out.results[c]["res_a"], dtype=np.float64)
            for j, idx in enumerate(sig_lists[c]):
                psum[idx] = ra[:, j].sum()
    return pg, pp, gg, psum, out


def _plan(pc, lab):
    sel = pc >= 0.5
    L1 = [int(i) for i in np.nonzero(sel & (lab == 1.0))[0]]
    L0 = [int(i) for i in np.nonzero(sel & (lab != 1.0))[0]]
    full_lists = [L1[c::N_CORES] for c in range(N_CORES)]
    sig_lists = [L0[c::N_CORES] for c in range(N_CORES)]
    K1 = max((len(x) for x in full_lists), default=0)
    K0 = max((len(x) for x in sig_lists), default=0)
    return L1, L0, full_lists, sig_lists, K1, K0


def kernel(predict_cls, predict_seg, labels, masks):
    pc = np.asarray(predict_cls, dtype=np.float64)
    lab = np.asarray(labels).astype(np.float64)

    # classification BCE (mean reduction) -- O(B), host
    eps = 1e-7
    pc_c = np.clip(pc, eps, 1.0 - eps)
    cls_loss = -np.mean(lab * np.log(pc_c) + (1.0 - lab) * np.log(1.0 - pc_c))

    L1, L0, full_lists, sig_lists, K1, K0 = _plan(pc, lab)
    n = float(len(L1) + len(L0))
    if n == 0.0:
        return (np.float32(cls_loss), np.float32(1e-4))

    seg16 = np.asarray(predict_seg).reshape(B, P, F).astype(np.float16)
    msk16 = np.asarray(masks).reshape(B, P, F).astype(np.float16)
    pg, pp, gg, psum, _ = _device_sums(seg16, msk16, full_lists, sig_lists, K1, K0)

    dice_sum = 0.0
    for i in L1:
        dice_sum += (2.0 * pg[i] + 1e-5) / (pp[i] + gg[i] + 1e-5)
    for i in L0:
        dice_sum += 25.0 / (psum[i] + 25.0)
    seg_loss = (n - dice_sum) / max(n, 1.0)
    return (np.float32(cls_loss), np.float32(seg_loss))
